# revision 35
# baseline (speedup 1.0000x reference)
"""8-core TRN2 Bass kernel, head-sharded (tensor-parallel pairs).

core i -> batch b=i//2, head-half hh=i%2 (8 of 16 heads).  Each core runs
QKV+RoPE+SDPA for its 8 heads over ALL 2048 tokens of its batch (no
duplicated K/V work).  Normalized y is written twice into a slot buffer,
once scaled by mask m_a=(hh==0) and once by m_b=(hh==1) (masks ride the
denominator-broadcast matmuls, so the masking is free on DVE), and each
pair runs 4 pipelined 2-rank ReduceScatter(add) collectives: every core
receives y of ALL 16 heads at ITS 1024-token half, in global-head slot
order — one uniform program, no core-dependent control flow.  Phase 3
projects the gathered y with a host-permuted w_proj slab.

All GEMMs bf16; softmax denominators on the tensor engine (ones-matmul
PSUM chains at partition rows 0/32/64); 1/x = exp(-ln(x)) on scalar.
"""

import numpy as np
from contextlib import ExitStack

import concourse.bass as bass
import concourse.tile as tile
from concourse import mybir
from concourse.bass import ts

import bass_rust


def _split_multi_waits(nc, max_waits=1):
    n = 0
    for fn in nc.m.functions:
        for blk in fn.blocks:
            insts = blk.instructions
            i = 0
            while i < len(insts):
                inst = insts[i]
                si = inst.sync_info
                waits = list(si.on_wait) if (si is not None and si.on_wait) else []
                if len(waits) > max_waits:
                    si.on_wait = waits[:max_waits]
                    extra = waits[max_waits:]
                    for j in range(0, len(extra), max_waits):
                        nop = mybir.InstNoOp(
                            name=nc.get_next_instruction_name(), ins=[], outs=[])
                        nop.engine = inst.engine
                        nop.sync_info = bass_rust.SyncInfo(
                            on_wait=extra[j:j + max_waits], on_update=[])
                        nc.register_instruction(nop, overwrite=True)
                        insts.insert(i, nop)
                        i += 1
                        n += 1
                i += 1
    return n


def _patched_drain_and_barrier(self, tick_clock, wait_clock):
    from concourse.vector_clock import ScopedClock
    nc = self.nc
    probe = nc.sync.nop()
    wait_clock.add_sem_waits(probe.ins, ScopedClock({None: tick_clock.global_clock}))
    si = probe.ins.sync_info
    waits = list(si.on_wait or []) if si is not None else []
    if len(waits) > 1:
        si.on_wait = [waits[0]]
        for w in waits[1:]:
            nop = nc.sync.nop()
            nsi = nop.ins.sync_info
            if nsi is None:
                nop.ins.sync_info = bass_rust.SyncInfo(on_wait=[w], on_update=[])
            else:
                nsi.on_wait = [w]
    nc.sync.drain()
    nc.all_engine_barrier()
    assert self.sems is not None
    popped = nc._tile_sem_poison_stack.pop()
    assert popped is self._sem_poison
    nc.clear_and_free_semaphores(list(self.sems.allocated().values()))
    nc.all_engine_barrier()


_patched = False


def _apply_patches():
    global _patched
    if not _patched:
        tile.TileContext._drain_and_barrier = _patched_drain_and_barrier
        _patched = True


F32R = mybir.dt.float32r
F32 = mybir.dt.float32
BF16 = mybir.dt.bfloat16
EXP = mybir.ActivationFunctionType.Exp
LN = mybir.ActivationFunctionType.Ln

B, T, D, H, HD = 4, 2048, 2048, 16, 128
CC = D // 128
MYH = 8                  # heads per core
NT = T                   # tokens per core (q and kv)
KC = NT // 128
NOUT = 1024              # output token rows per core
SCALE = 1.0 / float(np.sqrt(HD))
N_CORES = 8
RG = [[0, 1], [2, 3], [4, 5], [6, 7]]


def build_nc(n_cores=N_CORES):
    _apply_patches()
    nc = bass.Bass("TRN2", target_bir_lowering=False, debug=False,
                   num_devices=n_cores)
    xT = nc.dram_tensor("xT", [D, NT], BF16, kind="ExternalInput").ap()
    wqs = nc.dram_tensor("wqs", [MYH, 128, CC * 128], BF16, kind="ExternalInput").ap()
    wks = nc.dram_tensor("wks", [MYH, 128, CC * 128], BF16, kind="ExternalInput").ap()
    wvs = nc.dram_tensor("wvs", [2, 4, 128, 4 * 512], BF16, kind="ExternalInput").ap()
    wps = nc.dram_tensor("wps", [4, 4, 128, 4 * 512], BF16, kind="ExternalInput").ap()
    cs2 = nc.dram_tensor("cs2", [128, NT], BF16, kind="ExternalInput").ap()
    sn2 = nc.dram_tensor("sn2", [128, NT], BF16, kind="ExternalInput").ap()
    onesd = nc.dram_tensor("onesd", [128, 128], BF16, kind="ExternalInput").ap()
    msks = nc.dram_tensor("msks", [2, 128], BF16, kind="ExternalInput").ap()
    out = nc.dram_tensor("out", [NOUT, D], F32, kind="ExternalOutput").ap()

    qTs = nc.dram_tensor("qTs", [MYH * 128, NT], BF16).ap()
    kTs = nc.dram_tensor("kTs", [MYH * 128, NT], BF16).ap()
    vsc = nc.dram_tensor("vsc", [MYH, 128, KC * 128], BF16).ap()
    # exchange buffers: ysrc[g] = [half, slot(4 global heads), 128, 1024];
    # slots 0,1 = rank0's heads 2g,2g+1; slots 2,3 = rank1's — each core
    # fills ALL slots, scaled by its 0/1 mask, so ReduceScatter(add) yields
    # ydst[g] = [slot, 128, 1024] = all 4 heads at this core's token half.
    ysrc = nc.dram_tensor("ysrc", [4, 2, 4, 128, NOUT], BF16).ap()
    ydst = nc.dram_tensor("ydst", [4, 4, 128, NOUT], BF16).ap()

    xT_r = xT.rearrange("(cc p) t -> cc p t", p=128)

    with tile.TileContext(nc) as tc, ExitStack() as octx:
        one_pool = octx.enter_context(tc.tile_pool(name="one", bufs=1))
        ones128 = one_pool.tile([128, 1], BF16, tag="o128")
        nc.sync.dma_start(ones128[:], onesd[:, 0:1])
        mka = one_pool.tile([1, 128], BF16, tag="mka")
        nc.sync.dma_start(mka[:], msks[0:1, :])
        mkb = one_pool.tile([1, 128], BF16, tag="mkb")
        nc.sync.dma_start(mkb[:], msks[1:2, :])

        qh_pool = octx.enter_context(tc.tile_pool(name="qh", bufs=2))
        kh_pool = octx.enter_context(tc.tile_pool(name="kh", bufs=2))
        vh_pool = octx.enter_context(tc.tile_pool(name="vh", bufs=2))
        e_pool = octx.enter_context(tc.tile_pool(name="eT", bufs=4))
        rs_pool = octx.enter_context(tc.tile_pool(name="rs", bufs=2))
        yev_pool = octx.enter_context(tc.tile_pool(name="yev", bufs=4))
        es_pool = octx.enter_context(tc.tile_pool(name="es", bufs=3))

        # ---------------- phase 1: QKV + RoPE (my 8 heads, all tokens) ---
        with ExitStack() as p1:
            cs_pool = p1.enter_context(tc.tile_pool(name="cs", bufs=1))
            cs_sb = cs_pool.tile([128, NT], BF16, tag="cs")
            sn_sb = cs_pool.tile([128, NT], BF16, tag="sn")
            nc.sync.dma_start(cs_sb[:], cs2[:])
            nc.sync.dma_start(sn_sb[:], sn2[:])

            xt_pool = p1.enter_context(tc.tile_pool(name="xt", bufs=16))
            wqk_pool = p1.enter_context(tc.tile_pool(name="wqk", bufs=3))
            wv_pool = p1.enter_context(tc.tile_pool(name="wv", bufs=4))
            ev_pool = p1.enter_context(tc.tile_pool(name="ev", bufs=4))
            evv_pool = p1.enter_context(tc.tile_pool(name="evv", bufs=4))
            rp_pool = p1.enter_context(tc.tile_pool(name="rp", bufs=2))
            ps1 = p1.enter_context(tc.tile_pool(name="ps1", bufs=4, space="PSUM"))

            def rope_evict(ps, toff, dst):
                sf = rp_pool.tile([128, 512], F32, tag="sf")
                nc.scalar.copy(sf[:], ps[:])
                sw = rp_pool.tile([128, 512], F32, tag="sw")
                nc.gpsimd.dma_start(sw[0:64, :], sf[64:128, :])
                nc.gpsimd.dma_start(sw[64:128, :], sf[0:64, :])
                ta = rp_pool.tile([128, 512], F32, tag="ta")
                nc.vector.tensor_mul(ta[:], sf[:], cs_sb[:, toff:toff + 512])
                tb = rp_pool.tile([128, 512], F32, tag="tb")
                nc.vector.tensor_mul(tb[:], sw[:], sn_sb[:, toff:toff + 512])
                o = ev_pool.tile([128, 512], BF16, tag="ev")
                with nc.allow_low_precision(reason="bf16 eviction"):
                    nc.vector.tensor_add(o[:], ta[:], tb[:])
                nc.gpsimd.dma_start(dst, o[:])

            xq = []
            for cc in range(CC):
                t_ = xt_pool.tile([128, NT], BF16, tag="xt")
                nc.sync.dma_start(t_[:], xT_r[cc])
                xq.append(t_)

            def qk_head(h, wsrc, dstT):
                wsl = wqk_pool.tile([128, CC, 128], BF16, tag="wqk")
                nc.sync.dma_start(wsl[:], wsrc[h])
                for tt in range(NT // 512):
                    ps = ps1.tile([128, 512], F32, tag="ps1")
                    for cc in range(CC):
                        nc.tensor.matmul(ps[:], wsl[:, cc, :],
                                         xq[cc][:, ts(tt, 512)],
                                         start=(cc == 0), stop=(cc == CC - 1))
                    rope_evict(ps, tt * 512,
                               dstT[h * 128:(h + 1) * 128,
                                    tt * 512:(tt + 1) * 512])

            def v_group(ft):
                wvl = []
                for qt in range(4):
                    w_ = wv_pool.tile([128, 4, 512], BF16, tag="wv")
                    nc.sync.dma_start(w_[:], wvs[ft, qt])
                    wvl.append(w_)
                for tch in range(KC):
                    ps = ps1.tile([128, 512], F32, tag="ps1")
                    for cc in range(CC):
                        wv_ap = wvl[cc // 4][:, cc % 4, :]
                        nc.tensor.matmul(ps[:], xq[cc][:, ts(tch, 128)], wv_ap,
                                         start=(cc == 0), stop=(cc == CC - 1))
                    o = evv_pool.tile([128, 512], BF16, tag="evv")
                    nc.scalar.copy(o[:], ps[:])
                    for hh in range(4):
                        nc.gpsimd.dma_start(
                            vsc[ft * 4 + hh, :, tch * 128:(tch + 1) * 128],
                            o[:, hh * 128:(hh + 1) * 128])

            for h in range(MYH):
                qk_head(h, wqs, qTs)
            for h in range(0, 4):
                qk_head(h, wks, kTs)
            v_group(0)
            for h in range(4, MYH):
                qk_head(h, wks, kTs)
            v_group(1)

        # ---------------- phases 2+3 ----------------
        with ExitStack() as p23:
            wp_pool = p23.enter_context(tc.tile_pool(name="wp", bufs=6))
            wq3_pool = p23.enter_context(tc.tile_pool(name="wq3", bufs=4))
            outev_pool = p23.enter_context(tc.tile_pool(name="outev", bufs=4))
            yg_pool = p23.enter_context(tc.tile_pool(name="yg", bufs=1))
            wpl0 = []
            for qt in range(4):
                w_ = wp_pool.tile([128, 4, 512], BF16, tag="wp")
                nc.scalar.dma_start(w_[:], wps[0, qt])
                wpl0.append(w_)
            wq3 = {}
            for ft in range(4):
                w3_ = wq3_pool.tile([128, 4, 512], BF16, tag="wq3")
                nc.scalar.dma_start(w3_[:], wps[ft, 3])
                wq3[ft] = w3_
            # gathered y: slot order (g, s) -> global head (s//2)*8 + 2g + s%2
            # one tile per group so early out-proj chains don't wait on the
            # last group's ReduceScatter
            yg_tiles = []
            for g in range(4):
                ygt = yg_pool.tile([128, 4, NOUT], BF16, tag=f"yg{g}",
                                   name=f"ygt{g}")
                yg_tiles.append(ygt)

            # ----- phase 2: SDPA over my 8 heads -----
            with ExitStack() as p2:
                s_ps_pool = p2.enter_context(
                    tc.tile_pool(name="sps", bufs=2, space="PSUM"))
                o_ps_pool = p2.enter_context(
                    tc.tile_pool(name="ops", bufs=2, space="PSUM"))
                red_pool = p2.enter_context(
                    tc.tile_pool(name="red", bufs=1, space="PSUM"))
                bc_pool = p2.enter_context(
                    tc.tile_pool(name="bc", bufs=1, space="PSUM"))
                red_ps = red_pool.tile([96, 512], F32, tag="red")

                pendA = []
                pendB = []
                pendB2 = []
                pendM = []

                def emit_merge():
                    # fold the DVE bf16 denominator partial into a psum row
                    # (emitted one block later, when esv is long complete)
                    if not pendM:
                        return
                    row, esv = pendM.pop(0)
                    nc.tensor.matmul(red_ps[row:row + 1, :], ones128[:],
                                     esv[:, 0:512], start=True, stop=False)
                    nc.tensor.matmul(red_ps[row:row + 1, :], ones128[:],
                                     esv[:, 512:1024], start=False, stop=True)

                def emit_stageA():
                    if not pendA:
                        return
                    h, qh, row, o_ps = pendA.pop(0)
                    lnt = rs_pool.tile([1, 512], F32R, tag="lnt")
                    with nc.allow_low_precision(reason="f32r is 4-byte"):
                        nc.scalar.activation(lnt[:], red_ps[row:row + 1, :], LN)
                        rs = rs_pool.tile([1, 512], BF16, tag="rs")
                        nc.scalar.activation(rs[:], lnt[:], EXP, scale=-1.0)
                    pendB.append((h, qh, rs, o_ps))

                def _masked_evict(h, qh, rs, o_ps, slot_base, mk, tag):
                    # masked reciprocal broadcast (m/den) + evict one copy
                    g, ph = h // 2, h % 2
                    half, col = qh // 2, (qh % 2) * 512
                    bc_ps = bc_pool.tile([128, 512], F32, tag="bc")
                    nc.tensor.matmul(bc_ps[:, :], mk[:], rs[:],
                                     start=True, stop=True)
                    rcp = rs_pool.tile([128, 512], F32R, tag="rcp")
                    yev = yev_pool.tile([128, 512], BF16, tag=tag)
                    with nc.allow_low_precision(reason="f32r is 4-byte"):
                        nc.vector.tensor_copy(rcp[:], bc_ps[:])
                        nc.vector.tensor_mul(yev[:], o_ps[:], rcp[:])
                    nc.gpsimd.dma_start(
                        ysrc[g, half, slot_base + ph, :, col:col + 512],
                        yev[:])

                def emit_stageB():
                    if not pendB:
                        return
                    h, qh, rs, o_ps = pendB.pop(0)
                    _masked_evict(h, qh, rs, o_ps, 0, mka, "ya")
                    pendB2.append((h, qh, rs, o_ps))

                def emit_stageB2():
                    if not pendB2:
                        return
                    h, qh, rs, o_ps = pendB2.pop(0)
                    _masked_evict(h, qh, rs, o_ps, 2, mkb, "yb")
                    # after both heads of group g wrote all 4 q-chunks,
                    # fire the pair ReduceScatter for that group
                    g, ph = h // 2, h % 2
                    if ph == 1 and qh == (NT // 512) - 1:
                        nc.gpsimd.collective_compute(
                            "ReduceScatter", mybir.AluOpType.add,
                            replica_groups=RG,
                            ins=[ysrc[g].opt()], outs=[ydst[g].opt()])
                        for s in range(4):
                            nc.sync.dma_start(
                                yg_tiles[g][:, s, :], ydst[g, s])

                qcnt = 0
                for h in range(MYH):
                    qh_sb = qh_pool.tile([128, NT], BF16, tag="qh")
                    nc.sync.dma_start(qh_sb[:], qTs[h * 128:(h + 1) * 128, :])
                    kh_sb = kh_pool.tile([128, NT], BF16, tag="kh")
                    nc.sync.dma_start(kh_sb[:], kTs[h * 128:(h + 1) * 128, :])
                    vh_sb = vh_pool.tile([128, KC * 128], BF16, tag="vh")
                    nc.sync.dma_start(vh_sb[:], vsc[h])
                    for qh in range(NT // 512):
                        emit_stageB()
                        emit_merge()
                        emit_stageA()
                        qsl = qh_sb[:, ts(qh, 512)]
                        row = (qcnt % 3) * 32
                        qcnt += 1
                        o_ps = o_ps_pool.tile([128, 512], F32, tag="ops")
                        esv = es_pool.tile([128, 1024], BF16, tag="esv")
                        NP = KC // 2
                        eTs = [None] * NP

                        def es_add(j):
                            # pure-bf16 partial denominator on DVE (even
                            # tiles); folded into the psum chain at the end
                            with nc.allow_low_precision(reason="bf16 denom"):
                                if j == 0:
                                    nc.vector.tensor_copy(esv[:], eTs[j][:])
                                else:
                                    nc.vector.tensor_add(esv[:], esv[:],
                                                         eTs[j][:])

                        def pv_pair(j, last):
                            for u in range(2):
                                kc = 2 * j + u
                                nc.tensor.matmul(
                                    o_ps[:], vh_sb[:, ts(kc, 128)],
                                    eTs[j][:, ts(u, 512)],
                                    start=(kc == 0), stop=(last and u == 1))

                        for j in range(NP):
                            s_ps = s_ps_pool.tile([128, 1024], F32, tag="sps")
                            nc.tensor.matmul(s_ps[:, 0:512],
                                             kh_sb[:, ts(2 * j, 128)], qsl,
                                             start=True, stop=True)
                            nc.tensor.matmul(s_ps[:, 512:1024],
                                             kh_sb[:, ts(2 * j + 1, 128)], qsl,
                                             start=True, stop=True)
                            eT = e_pool.tile([128, 1024], BF16, tag="eT")
                            nc.scalar.activation(eT[:], s_ps[:], EXP, scale=SCALE)
                            eTs[j] = eT
                            if j == 1:
                                emit_stageB2()
                            if j >= 1:
                                es_add(j - 1)
                            if j >= 2:
                                pv_pair(j - 2, last=False)
                        es_add(NP - 1)
                        pv_pair(NP - 2, last=False)
                        pv_pair(NP - 1, last=True)
                        pendM.append((row, esv))
                        pendA.append((h, qh, row, o_ps))
                while pendA or pendB or pendB2 or pendM:
                    emit_merge()
                    emit_stageA()
                    emit_stageB()
                    emit_stageB2()

            # ----- phase 3: output projection over gathered y -----
            # two passes: groups 0-2 accumulate into SBUF partials while the
            # last group's ReduceScatter is still in flight; group 3's
            # contribution is chained in pass 2 and folded in on the vector
            # engine during eviction.
            with ExitStack() as p3:
                ps3 = p3.enter_context(tc.tile_pool(name="ps3", bufs=4, space="PSUM"))
                po_pool = p3.enter_context(tc.tile_pool(name="po", bufs=32))

                po = {}
                for ft in range(4):
                    if ft == 0:
                        wpl = wpl0
                    else:
                        wpl = []
                        for qt in range(3):
                            w_ = wp_pool.tile([128, 4, 512], BF16, tag="wp")
                            nc.scalar.dma_start(w_[:], wps[ft, qt])
                            wpl.append(w_)
                    for tch in range(NOUT // 128):
                        ps = ps3.tile([128, 512], F32, tag="ps3")
                        for hc in range(12):
                            wp_ap = wpl[hc // 4][:, hc % 4, :]
                            nc.tensor.matmul(
                                ps[:], yg_tiles[hc // 4][:, hc % 4, ts(tch, 128)],
                                wp_ap, start=(hc == 0), stop=(hc == 11))
                        p_ = po_pool.tile([128, 512], F32, tag="po")
                        nc.scalar.copy(p_[:], ps[:])
                        po[(ft, tch)] = p_
                for ft in range(4):
                    for tch in range(NOUT // 128):
                        ps = ps3.tile([128, 512], F32, tag="ps3")
                        for s in range(4):
                            nc.tensor.matmul(
                                ps[:], yg_tiles[3][:, s, ts(tch, 128)],
                                wq3[ft][:, s, :],
                                start=(s == 0), stop=(s == 3))
                        oev = outev_pool.tile([128, 512], F32, tag="outev")
                        nc.vector.tensor_add(oev[:], ps[:], po[(ft, tch)][:])
                        nc.gpsimd.dma_start(
                            out[ts(tch, 128), ft * 512:(ft + 1) * 512], oev[:])

    _split_multi_waits(nc)
    return nc


# ---------------------------------------------------------------------------
# host-side prep / assembly
# ---------------------------------------------------------------------------


def prep_inputs(x, w_attn, w_proj):
    from ml_dtypes import bfloat16
    x = np.asarray(x, dtype=np.float32)
    w_attn = np.asarray(w_attn, dtype=np.float32)
    w_proj = np.asarray(w_proj, dtype=np.float32)

    perm = np.concatenate([np.arange(0, HD, 2), np.arange(1, HD, 2)])
    colperm = (np.arange(H)[:, None] * HD + perm[None, :]).ravel()

    wq, wk, wv = w_attn[0:D], w_attn[D:2 * D], w_attn[2 * D:3 * D]
    wqs_f = np.ascontiguousarray(
        wq.T[:, colperm].reshape(CC, 128, H, 128)
        .transpose(2, 1, 0, 3).reshape(H, 128, CC * 128)).astype(bfloat16)
    wks_f = np.ascontiguousarray(
        wk.T[:, colperm].reshape(CC, 128, H, 128)
        .transpose(2, 1, 0, 3).reshape(H, 128, CC * 128)).astype(bfloat16)
    wvs_f = np.ascontiguousarray(
        wv.T.reshape(4, 4, 128, 4, 512)
        .transpose(3, 0, 2, 1, 4).reshape(4, 4, 128, 4 * 512)).astype(bfloat16)
    # w_proj: permute input-channel head blocks into the gather slot order
    # (g, s): global head (s//2)*8 + 2g + s%2, then pack as usual
    gather_heads = [(s // 2) * 8 + 2 * g + (s % 2)
                    for g in range(4) for s in range(4)]
    wpT_perm = w_proj.T.reshape(H, 128, D)[gather_heads].reshape(D, D)
    wps_f = np.ascontiguousarray(
        wpT_perm.reshape(4, 4, 128, 4, 512)
        .transpose(3, 0, 2, 1, 4).reshape(4, 4, 128, 4 * 512)).astype(bfloat16)

    inv = 1.0 / (10000.0 ** (np.arange(0, HD, 2, dtype=np.float64) / HD))
    fr = np.outer(np.arange(T, dtype=np.float64), inv)
    cs2 = np.concatenate([np.cos(fr).T, np.cos(fr).T], 0).astype(bfloat16)
    sn2 = np.concatenate([-np.sin(fr).T, np.sin(fr).T], 0).astype(bfloat16)

    onesb = np.ones((128, 128), dtype=bfloat16)

    in_maps = []
    for i in range(N_CORES):
        b, hh = i // 2, i % 2
        msks = np.zeros((2, 128), dtype=bfloat16)
        msks[hh, :] = 1
        in_maps.append({
            "xT": np.ascontiguousarray(x[b].T).astype(bfloat16),
            "wqs": np.ascontiguousarray(wqs_f[hh * MYH:(hh + 1) * MYH]),
            "wks": np.ascontiguousarray(wks_f[hh * MYH:(hh + 1) * MYH]),
            "wvs": np.ascontiguousarray(wvs_f[2 * hh:2 * hh + 2]),
            "wps": wps_f,
            "cs2": cs2, "sn2": sn2,
            "onesd": onesb,
            "msks": msks,
        })
    return in_maps


def assemble(results):
    out = np.empty((B, T, D), dtype=np.float32)
    for i in range(N_CORES):
        b, hh = i // 2, i % 2
        out[b, hh * NOUT:(hh + 1) * NOUT, :] = results[i]["out"]
    return out


_nc_cache = None


def _get_nc():
    global _nc_cache
    if _nc_cache is None:
        _nc_cache = build_nc()
    return _nc_cache


def kernel(x, w_attn, w_proj):
    from concourse.bass_utils import run_bass_kernel_spmd
    nc = _get_nc()
    in_maps = prep_inputs(x, w_attn, w_proj)
    res = run_bass_kernel_spmd(nc, in_maps, list(range(N_CORES)))
    return assemble(res.results)


def run_profiled(x, w_attn, w_proj, trace_cores=None):
    from concourse.bass_utils import run_bass_kernel_spmd
    import sys as _sys, types as _types
    try:
        import antenv
        if "antenv.axon_hooks" not in _sys.modules:
            mod = _types.ModuleType("antenv.axon_hooks")
            _h = [None]
            mod.set_axon_ntff_profile_hook = lambda h: _h.__setitem__(0, h)
            mod.get_axon_ntff_profile_hook = lambda: _h[0]
            _sys.modules["antenv.axon_hooks"] = mod
            antenv.axon_hooks = mod
            from trn_agent_boot.trn_boot import _ntff_profile_via_ctypes
            mod.set_axon_ntff_profile_hook(
                _ntff_profile_via_ctypes('/opt/axon/libaxon_pjrt.so'))
    except Exception as e:
        print("profile hook setup failed:", e)
    nc = _get_nc()
    in_maps = prep_inputs(x, w_attn, w_proj)
    return run_bass_kernel_spmd(
        nc, in_maps, list(range(N_CORES)), trace=True,
        trace_cores=trace_cores if trace_cores is not None else [0])
